# revision 48
# baseline (speedup 1.0000x reference)
"""Trainium2 Bass kernel for nn_DecoderLayer (self-attn + cross-attn + FFN).

Sharding: 8 cores = 4 batch x 2 interleaved query-block sets (no collectives).
Core (b, th) handles query blocks {2j+th : j=0..3} of batch b, computing
full-sequence K/V locally (feature-major layout, fp16 matmuls, fp32
accumulation; softmax/LN plumbing off the PE).

Structure (v2):
- causal mask applied as a post-exp 0/1 multiply on DVE, trimmed to the one
  diagonal query block per key block (no PE mask matmuls); interleaved query
  blocks balance causal work across the th pair
- attention AV uses V as the stationary operand, producing feature-major
  output directly (no PE transposes); the softmax denominator rides as an
  appended ones-column of V; normalization via DVE reciprocal_approx_fast +
  gpsimd partition_broadcast + DVE multiply
- layernorm stats via DVE tree-adds + gpsimd partition_all_reduce (no PE
  ones-matmuls); rstd via scalar Sqrt + DVE reciprocal_approx_fast
- self-attention emission interleaves cross-attn K/V projection blocks so
  the PE stays busy while the scalar engine grinds exp
- residual stream kept in fp16 (out1/out2), fp16 output DMA
"""

import sys

for _p in ("/opt/trn_rl_repo",):
    if _p not in sys.path:
        sys.path.insert(0, _p)

import numpy as np

import concourse.bass as bass
from concourse import bacc, bass_isa
import concourse.mybir as mybir
import concourse.tile as tile

T, S, B, D, H, DH, DI = 1024, 1024, 4, 1024, 16, 64, 4096
P = 128
NQ = T // 2          # queries per core
DC = D // P          # 8  d chunks
KC = T // P          # 8  key chunks (self)
EC = S // P          # 8  key chunks (cross)
QC = NQ // P         # 4  query chunks
HC = (H * DH) // P   # 8  head-feature chunks
FIC = DI // P        # 32 ffn inner chunks
SCALE = 1.0 / (DH ** 0.5)
EPS = 1e-5

F16 = mybir.dt.float16
F32 = mybir.dt.float32
AF = mybir.ActivationFunctionType
OP = mybir.AluOpType
RED = bass_isa.ReduceOp

_CACHE = {}


def _build(debug=False, ec_lim=EC, self_skip=True, emask0=True):
    nc = bacc.Bacc()

    dx = nc.dram_tensor("x_fm", [D, T], F16, kind="ExternalInput")
    dxq = nc.dram_tensor("xq_fm", [D, NQ], F16, kind="ExternalInput")
    denc = nc.dram_tensor("enc_fm", [D, S], F16, kind="ExternalInput")
    mask_w = P if self_skip else NQ
    dmask = nc.dram_tensor("maskT", [T, mask_w], F16, kind="ExternalInput")
    demask = nc.dram_tensor("emask", [P, EC], F32, kind="ExternalInput")
    # proj_cols weights arrive pre-tiled per output block fc: [n_fc*P, 8*P]
    # (row fc*P+p, col dc*P+q) = W[dc*P+p, fc*P+q]  -> one contiguous 2KB
    # DMA line per partition.  wv1/wv2 stay row-major (proj_v reads rows).
    PAIR = [HC // 2 * P, 2 * DC * P]      # proj_cols pair-tiled weights
    dw = {}
    for nm, shp in [
        ("wq1", PAIR), ("wk1", PAIR), ("wv1", [D, H * DH]),
        ("wo1", PAIR),
        ("wq2", PAIR), ("wk2", PAIR), ("wv2", [D, H * DH]),
        ("wo2", PAIR),
        ("wff1", [FIC // 2 * P, 2 * DC * P]),
        ("wff2", [DC * 2 * P, (FIC // 2) * P]),
    ]:
        dw[nm] = nc.dram_tensor(nm, shp, F16, kind="ExternalInput")
    # g1/b1 and g3/b3 are folded into the adjacent weights host-side:
    # wk1/wv1/wff1 rows are pre-scaled by g, b1's effect on self-attn k
    # cancels in softmax, its effect on v is the fixed vector d1 added at
    # the Wo1 writer, and b3 folds into the gelu bias bf1.
    dln = {}
    for nm, w in [("g2", DC), ("b2", DC), ("bf1", FIC), ("bf2", DC),
                  ("d1", DC)]:
        dln[nm] = nc.dram_tensor(nm, [P, w], F32, kind="ExternalInput")
    dout = nc.dram_tensor("out_fm", [D, NQ], F16, kind="ExternalOutput")

    with tile.TileContext(nc) as tc:
        with tc.tile_pool(name="sb", bufs=1) as sb, \
             tc.tile_pool(name="pp", bufs=2, space="PSUM") as pp:

            # ---------------- constants ----------------
            epsP = sb.tile([P, 1], F32, tag="epsP", name="epsP")
            nc.vector.memset(epsP, EPS)
            ones1 = sb.tile([P, 1], F16, tag="ones1", name="ones1")
            nc.vector.memset(ones1, 1.0)
            onesr = sb.tile([1, P], F16, tag="onesr", name="onesr")
            nc.vector.memset(onesr, 1.0)
            ln = {}
            for nm in dln:
                w = FIC if nm == "bf1" else DC
                ln[nm] = sb.tile([P, w], F32, tag=nm, name=nm)
                nc.sync.dma_start(ln[nm], dln[nm][:, :])
            emask_sb = sb.tile([P, EC], F32, tag="emask", name="emask")
            nc.sync.dma_start(emask_sb, demask[:, :])

            def fam(prefix, n, shape, dtype):
                return [sb.tile(shape, dtype, tag=f"{prefix}{i}", name=f"{prefix}{i}")
                        for i in range(n)]

            def dbg(name, tiles):
                if not debug:
                    return
                w = tiles[0].shape[-1] if len(tiles[0].shape) == 2 else (
                    tiles[0].shape[1] * tiles[0].shape[2])
                dt_ = nc.dram_tensor(name, [len(tiles) * P, w], tiles[0].dtype,
                                     kind="ExternalOutput")
                r = dt_.rearrange("(c p) t -> p c t", p=P)
                for i, t_ in enumerate(tiles):
                    if len(t_.shape) == 3:
                        t_ = t_.rearrange("p a b -> p (a b)")
                    nc.sync.dma_start(r[:, i, :], t_)

            # ---------------- feature-major layernorm ----------------
            def ln_pe(src, blocks, out16, g=None, b=None, name="ln"):
                """LN over the partition (feature) dim; src: DC f16 tiles.

                Stats via PE ones-matmuls (squares on DVE), mean/rstd rows
                broadcast across partitions via K=1 matmuls, short DVE
                apply.  g=None emits (x-m)*rstd only — the gain is folded
                into the consumer's weights host-side.  blocks: list of
                (t0, t1) token windows, each at most 512 wide.
                """
                for j, (jt0, jt1) in enumerate(blocks):
                    js = slice(jt0, jt1)
                    W = jt1 - jt0
                    st = pp.tile([P, 512], F32, tag="avb", name="lns1", bufs=2)
                    st2 = pp.tile([P, 512], F32, tag="avb", name="lns2", bufs=2)
                    for dc in range(DC):
                        sq = sb.tile([P, 512], F16, tag="lnsq", name="lnsq",
                                     bufs=2)
                        nc.vector.tensor_mul(sq[:, :W], src[dc][:, js],
                                             src[dc][:, js])
                        nc.tensor.matmul(st[0:1, :W], lhsT=ones1,
                                         rhs=src[dc][:, js],
                                         start=(dc == 0), stop=(dc == DC - 1))
                        nc.tensor.matmul(st2[0:1, :W], lhsT=ones1,
                                         rhs=sq[:, :W],
                                         start=(dc == 0), stop=(dc == DC - 1))

                    # stat rows as 32-aligned partition-slices of one
                    # scratch tile (slots reused once their reader is done)
                    lr = sb.tile([P, 512], F32, tag="lnr", name="lnr", bufs=2)
                    mm, m2t, var = (lr[0:1, :W], lr[32:33, :W], lr[64:65, :W])
                    nc.vector.tensor_scalar(mm, st[0:1, :W], 1.0 / D, 0.0,
                                            OP.mult, OP.add)
                    nc.vector.tensor_mul(m2t, mm, mm)
                    nc.vector.scalar_tensor_tensor(var, st2[0:1, :W], 1.0 / D,
                                                   m2t, OP.mult, OP.subtract)
                    # rstd = exp(-0.5*ln(var+eps)) — Log/Exp share one ACT
                    # table set (no Sqrt set switch, no DVE reciprocal)
                    lr2 = sb.tile([P, 512], F32, tag="lnr", name="lnr2",
                                  bufs=2)
                    lnv, rstd = lr2[32:33, :W], lr2[0:1, :W]
                    nc.scalar.activation(lnv, var, AF.Ln, bias=epsP[64:65])
                    nc.scalar.activation(rstd, lnv, AF.Exp, scale=-0.5)
                    r16 = sb.tile([1, 512], F16, tag="lnf", name="r16", bufs=2)
                    s16 = sb.tile([1, 512], F16, tag="lnf2", name="s16",
                                  bufs=2)  # -m*rstd (fold) or m (g path)
                    r16, s16 = r16[:, :W], s16[:, :W]
                    nc.vector.tensor_copy(r16, rstd)
                    if g is None:
                        nc.vector.scalar_tensor_tensor(s16, mm, -1.0, rstd,
                                                       OP.mult, OP.mult)
                    else:
                        nc.vector.tensor_copy(s16, mm)
                    bc = pp.tile([P, 1024], F32, tag="sc2", name="lnbc",
                                 bufs=2)
                    nc.tensor.matmul(bc[:, 0:W], lhsT=onesr, rhs=r16,
                                     start=True, stop=True)
                    nc.tensor.matmul(bc[:, 512:512 + W], lhsT=onesr, rhs=s16,
                                     start=True, stop=True)
                    rb = sb.tile([P, 512], F16, tag="lnrb", name="rb", bufs=2)
                    nc.vector.tensor_copy(rb[:, :W], bc[:, 0:W])
                    mb = sb.tile([P, 512], F16, tag="lnmb", name="mb", bufs=2)
                    nc.vector.tensor_copy(mb[:, :W], bc[:, 512:512 + W])
                    rb, mb = rb[:, :W], mb[:, :W]
                    for dc in range(DC):
                        if g is None:
                            t = sb.tile([P, 512], F16, tag="lnt", name="lnt",
                                        bufs=2)
                            nc.vector.tensor_mul(t[:, :W], src[dc][:, js], rb)
                            nc.vector.tensor_add(out16[dc][:, js], t[:, :W],
                                                 mb)
                        else:
                            t = sb.tile([P, 512], F16, tag="lnt", name="lnt",
                                        bufs=2)
                            nc.vector.tensor_sub(t[:, :W], src[dc][:, js], mb)
                            nc.vector.tensor_mul(t[:, :W], t[:, :W], rb)
                            nc.vector.tensor_scalar(
                                out16[dc][:, js], t[:, :W], g[:, dc:dc + 1],
                                b[:, dc:dc + 1], OP.mult, OP.add)

            # ---------------- generic column-block projection ------------
            def proj_cols(wd, rhs, n_fc, writer, rhs_w=NQ, closures=False,
                          jwin=None):
                wr = wd.rearrange("(fp p) x -> fp p x", p=P)
                j0lo, j0hi = jwin if jwin else (0, rhs_w)

                def emit(fp):
                    # one 4KB-per-partition DMA covers output blocks 2fp,2fp+1
                    wt = sb.tile([P, 2 * DC * P], F16, tag="wb", name="wb",
                                 bufs=3)
                    nc.sync.dma_start(wt, wr[fp])
                    for e in range(2):
                        fc = fp * 2 + e
                        for j0 in range(j0lo, j0hi, 512):
                            w_ = min(512, j0hi - j0)
                            js = slice(j0, j0 + w_)
                            acc = pp.tile([P, 512], F32, tag="acc", name="acc",
                                          bufs=2)
                            for dc in range(DC):
                                o = (e * DC + dc) * P
                                nc.tensor.matmul(
                                    acc[:, :w_], lhsT=wt[:, o:o + P],
                                    rhs=rhs[dc][:, js],
                                    start=(dc == 0), stop=(dc == DC - 1))
                            if jwin is None and rhs_w == 512:
                                writer(fc, acc)
                            else:
                                writer(fc, acc[:, :w_], js)

                if closures:
                    return [lambda fp=fp: emit(fp) for fp in range(n_fc // 2)]
                for fp in range(n_fc // 2):
                    emit(fp)

            def proj_v(wd, src, va, ntc=KC, closures=False):
                """token-major V projection (appended ones col), cached W."""
                wr = wd.rearrange("(dc p) f -> p dc f", p=P)
                wvt = []

                def load_w():
                    for dc in range(DC):
                        t = sb.tile([P, H * DH], F16, tag=f"wv{dc}",
                                    name=f"wv{dc}", bufs=1)
                        nc.sync.dma_start(t, wr[:, dc, :])
                        wvt.append(t)

                def emit(tc8):
                    accs = [pp.tile([P, 512], F32, tag="acc", name="acc", bufs=2)
                            for _ in range(2)]
                    for dc in range(DC):
                        for jn in range(2):
                            nc.tensor.matmul(
                                accs[jn],
                                lhsT=src[dc][:, tc8 * P:(tc8 + 1) * P],
                                rhs=wvt[dc][:, jn * 512:(jn + 1) * 512],
                                start=(dc == 0), stop=(dc == DC - 1))
                    for jn in range(2):
                        nc.vector.tensor_copy(
                            va[tc8][:, jn * (H // 2):(jn + 1) * (H // 2), 0:DH],
                            accs[jn].rearrange("p (h d) -> p h d", h=H // 2))
                    nc.gpsimd.memset(va[tc8][:, :, DH:DH + 1], 1.0)

                cl = [load_w] + [(lambda t=t: emit(t)) for t in range(ntc)]
                if closures:
                    return cl
                for c in cl:
                    c()

            # ---------------- attention ----------------
            def attention(qt, kt, va, vec, masked, filler=(), name="sa",
                          qwin=(0, NQ)):
                nkc = KC if masked else ec_lim
                trim = masked and self_skip
                qlo, qhi = qwin
                W = qhi - qlo
                assert not masked or (qlo, qhi) == (0, NQ)

                def q_start(kc):
                    return (kc // 2) * P if trim else 0

                filler = list(filler)
                done = [0]

                def run_filler(i):
                    want = ((i + 1) * len(filler)) // HC
                    while done[0] < want:
                        filler[done[0]]()
                        done[0] += 1

                dbg_av = dbg_den = None
                if debug:
                    dbg_av = nc.dram_tensor(f"dbg_{name}_av", [H * (DH + 1), 512],
                                            F16, kind="ExternalOutput")
                    dbg_den = nc.dram_tensor(f"dbg_{name}_den", [H, 512],
                                             F32, kind="ExternalOutput")

                # narrow (split) windows pack a kc PAIR per head-bank so
                # each bank is filled by two SERIAL same-row-group matmuls
                # (concurrent row-tiled matmuls must not share a psum bank)
                pair = W < 512

                def pcols(hh, q0):
                    # head hh's probs for queries [qlo+q0:qhi) sit at columns
                    # [q0:W) (hh0) / [W:2W-q0) (hh1) — hh1 is shifted left so
                    # the exp span [q0:2W-q0) is contiguous valid
                    return slice(W, 2 * W - q0) if hh else slice(q0, W)

                def prob_ap(pl, kc, hh):
                    if pair:
                        o = hh * 512 + (kc % 2) * W
                        return pl[kc // 2][:, o:o + W]
                    return pl[kc][:, pcols(hh, q_start(kc))]

                def emit_avs(fch, pl):
                    for hh in range(2):
                        h = fch * 2 + hh
                        row = hh * DH
                        av = pp.tile([DH + 1, 512], F32, tag="avb", name="avb",
                                     bufs=2)
                        for kc in range(nkc):
                            q0 = q_start(kc)
                            nc.tensor.matmul(
                                av[:, q0:W], lhsT=va[kc][:, h, :],
                                rhs=prob_ap(pl, kc, hh),
                                start=(kc == 0), stop=(kc == nkc - 1))
                        if debug:
                            cp = sb.tile([DH + 1, 512], F16, tag="dbgav",
                                         name="dbgav", bufs=1)
                            nc.vector.tensor_copy(cp, av)
                            nc.sync.dma_start(
                                dbg_av.rearrange("(h d) q -> h d q", h=H)[h], cp)
                        den_s = sb.tile([1, 512], F32, tag="dens", name="dens",
                                        bufs=2)
                        nc.vector.tensor_copy(den_s[:, :W], av[DH:DH + 1, :W])
                        den = sb.tile([1, 512], F32, tag="den", name="den",
                                      bufs=2)
                        nc.vector.reciprocal_approx_fast(den[:, :W],
                                                         den_s[:, :W])
                        if debug:
                            nc.sync.dma_start(dbg_den[h:h + 1, qlo:qhi],
                                              den[:, :W])
                        denb = sb.tile([DH, 512], F32, tag="denb", name="denb",
                                       bufs=2)
                        nc.gpsimd.partition_broadcast(denb[:, :W], den[:, :W],
                                                      channels=DH)
                        nc.vector.tensor_mul(vec[fch][row:row + DH, qlo:qhi],
                                             av[0:DH, :W], denb[:, :W])

                prev = None
                assert not pair or (not masked and emask0)
                for fch in range(HC):
                    pl = []
                    if pair:
                        for j0 in range(0, nkc, 2):
                            kcs = range(j0, min(j0 + 2, nkc))
                            sp = pp.tile([P, 1024], F32, tag="sc2",
                                         name="sc2", bufs=2)
                            for hh in range(2):
                                row = hh * DH
                                for i, kc in enumerate(kcs):
                                    nc.tensor.matmul(
                                        sp[:, hh * 512 + i * W:
                                           hh * 512 + (i + 1) * W],
                                        lhsT=kt[fch][row:row + DH,
                                                     kc * P:(kc + 1) * P],
                                        rhs=qt[fch][row:row + DH, qlo:qhi],
                                        start=(i == 0),
                                        stop=(i == len(kcs) - 1),
                                        tile_position=(row, 0))
                            pt = sb.tile([P, 1024], F16, tag="p", name="p",
                                         bufs=10)
                            hi = 512 + len(kcs) * W
                            nc.scalar.activation(pt[:, :hi], sp[:, :hi],
                                                 AF.Exp, scale=SCALE)
                            pl.append(pt)
                        if prev is not None:
                            emit_avs(*prev)
                        run_filler(fch)
                        prev = (fch, pl)
                        continue
                    for kc in range(nkc):
                        q0 = q_start(kc)
                        # both heads' scores in one 2-bank PSUM tile so exp
                        # runs as a single wide ACTIVATE (amortizes overhead)
                        sp = pp.tile([P, 1024], F32, tag="sc2", name="sc2",
                                     bufs=2)
                        for hh in range(2):
                            row = hh * DH
                            nc.tensor.matmul(
                                sp[:, pcols(hh, q0)],
                                lhsT=kt[fch][row:row + DH, kc * P:(kc + 1) * P],
                                rhs=qt[fch][row:row + DH, qlo + q0:qhi],
                                start=True, stop=True, tile_position=(row, 0))
                        pt = sb.tile([P, 1024], F16, tag="p", name="p",
                                     bufs=10)
                        if masked:
                            nc.scalar.activation(pt[:, q0:2 * W - q0],
                                                 sp[:, q0:2 * W - q0],
                                                 AF.Exp, scale=SCALE)
                            if trim:
                                for hh in range(2):
                                    o = hh * 512 + (q0 if hh == 0 else 0)
                                    nc.vector.tensor_mul(
                                        pt[:, o:o + P], pt[:, o:o + P],
                                        mask_sb[kc])
                            else:
                                for hh in range(2):
                                    o = hh * 512
                                    nc.vector.tensor_mul(
                                        pt[:, o:o + 512], pt[:, o:o + 512],
                                        mask_sb[kc])
                        elif emask0:
                            nc.scalar.activation(pt[:, :2 * W], sp[:, :2 * W],
                                                 AF.Exp, scale=SCALE)
                        else:
                            for hh in range(2):
                                o = hh * W
                                nc.scalar.activation(
                                    pt[:, o:o + W], sp[:, o:o + W], AF.Exp,
                                    bias=emask_sb[:, kc:kc + 1], scale=SCALE)
                        pl.append(pt)
                    if prev is not None:
                        emit_avs(*prev)
                    run_filler(fch)
                    prev = (fch, pl)
                emit_avs(*prev)

            # ================ phase A: load x, q1, LN1 ================
            xq_t = fam("q", DC, [P, NQ], F16)        # xq (q-proj rhs + residual)
            dxq_r = dxq.rearrange("(dc p) t -> p dc t", p=P)
            for dc in range(DC):
                nc.sync.dma_start(xq_t[dc], dxq_r[:, dc, :])

            t_t = fam("t", HC, [P, NQ], F16)         # q1, later h2_h
            # q1 only needs xq — emit before LN1 so PE is busy during LN1;
            # x is DMAed after q1's weights so q1 starts ASAP
            proj_cols(dw["wq1"], xq_t, HC,
                      lambda fc, acc: nc.vector.tensor_copy(t_t[fc], acc))

            e_t = fam("e", DC, [P, T], F16)          # x, then c, later enc
            dx_r = dx.rearrange("(dc p) t -> p dc t", p=P)
            for dc in range(DC):
                nc.sync.dma_start(e_t[dc], dx_r[:, dc, :])

            mask_sb = []
            dmask_r = dmask.rearrange("(kc p) q -> p kc q", p=P)
            for kc in range(KC):
                mt = sb.tile([P, mask_w], F16, tag=f"m{kc}", name=f"m{kc}")
                nc.sync.dma_start(mt, dmask_r[:, kc, :])
                mask_sb.append(mt)

            ln_pe(e_t, [(0, 512), (512, 1024)], out16=e_t, name='ln1')   # c' = (x-m)*rstd in e_t (g1 folded)
            dbg("dbg_c", e_t)

            # ================ phase B: self-attn K/V ================
            k_t = fam("k", HC, [P, T], F16)          # k1, later k2
            va_t = fam("va", KC, [P, H, DH + 1], F16)
            proj_cols(dw["wk1"], e_t, HC,
                      lambda fc, acc, js: nc.vector.tensor_copy(k_t[fc][:, js], acc),
                      rhs_w=T)
            proj_v(dw["wv1"], e_t, va_t)
            dbg("dbg_q1", t_t)
            dbg("dbg_k1", k_t)
            dbg("dbg_va", va_t)

            # enc + cross K/V are independent of self-attn; their projection
            # blocks run as PE filler between self-attn head groups.
            e2_t = fam("e", DC, [P, S], F16)
            denc_r = denc.rearrange("(dc p) t -> p dc t", p=P)
            for dc in range(DC):
                nc.sync.dma_start(e2_t[dc], denc_r[:, dc, :])
            k2_t = fam("k", HC, [P, S], F16)
            va2_t = fam("va", EC, [P, H, DH + 1], F16)
            k2_cl = proj_cols(
                dw["wk2"], e2_t, HC,
                lambda fc, acc, js: nc.vector.tensor_copy(k2_t[fc][:, js], acc),
                rhs_w=ec_lim * P, closures=True)
            v2_cl = proj_v(dw["wv2"], e2_t, va2_t, ntc=ec_lim, closures=True)
            # K2 (+ the V2 weight DMA) fills self-attention; the V2 emits run
            # later, under LN2/q2 where the PE would otherwise idle
            filler = [v2_cl[0]] + k2_cl

            # ================ phase C: self-attention ================
            vec_t = fam("s", HC, [P, NQ], F16)       # vec1, later vec2, h3
            attention(t_t, k_t, va_t, vec_t, masked=True, filler=filler, name="sa")
            dbg("dbg_vec", vec_t)

            # ================ phase D: Wo1 + residual, LN2 ================
            r_t = fam("r", DC, [P, NQ], F16)         # out1 (f16 residual src)
            proj_cols(dw["wo1"], vec_t, DC,
                      lambda fc, acc: nc.vector.scalar_tensor_tensor(
                          r_t[fc], acc, ln["d1"][:, fc:fc + 1], xq_t[fc],
                          OP.add, OP.add))
            for c in v2_cl[1:]:
                c()
            dbg("dbg_out1", r_t)

            h2h_t = fam("t", HC, [P, NQ], F16)       # reuse t family
            ln_pe(r_t, [(0, 512)], out16=h2h_t, g=ln["g2"], b=ln["b2"],
                  name='ln2')
            dbg("dbg_h2", h2h_t)

            # ====== phase E/F: cross-attention || LN3+FFN1 (query halves),
            # ====== then full-width FFN2
            q2_t = fam("q", HC, [P, NQ], F16)        # reuse q family
            proj_cols(dw["wq2"], h2h_t, HC,
                      lambda fc, acc: nc.vector.tensor_copy(q2_t[fc], acc))

            vec2_t = fam("s", HC, [P, NQ], F16)
            w_t = fam("w", DC, [P, NQ], F16)         # out2
            h3_t = fam("r", DC, [P, NQ], F16)        # reuse out1's slots
            g_t = fam("gg", FIC, [P, NQ], F16)       # (dead after LN2)
            halves = (((0, NQ // 2), (NQ // 2, NQ)) if emask0
                      else ((0, NQ),))
            for t0, t1 in halves:
                attention(q2_t, k2_t, va2_t, vec2_t, masked=False,
                          name=f"ca{t0}", qwin=(t0, t1))
                proj_cols(dw["wo2"], vec2_t, DC,
                          lambda fc, acc, js: nc.vector.tensor_add(
                              w_t[fc][:, js], acc, h2h_t[fc][:, js]),
                          jwin=(t0, t1))
                ln_pe(w_t, [(t0, t1)], out16=h3_t, name=f'ln3_{t0}')
                proj_cols(dw["wff1"], h3_t, FIC,
                          lambda fc, acc, js: nc.scalar.activation(
                              g_t[fc][:, js], acc, AF.Gelu,
                              bias=ln["bf1"][:, fc:fc + 1], scale=1.0),
                          jwin=(t0, t1))
            dbg("dbg_vec2", vec2_t)
            dbg("dbg_out2", w_t)

            dout_r = dout.rearrange("(dc p) q -> p dc q", p=P)
            w2r = dw["wff2"].rearrange("(dc half p) x -> dc half p x",
                                       half=2, p=P)
            for dc in range(DC):
                acc = pp.tile([P, 512], F32, tag="acc", name="acc", bufs=2)
                for half in range(2):
                    w2t = sb.tile([P, (FIC // 2) * P], F16, tag="wf2",
                                  name="wf2", bufs=2)
                    nc.sync.dma_start(w2t, w2r[dc, half])
                    for f in range(FIC // 2):
                        fic = half * (FIC // 2) + f
                        nc.tensor.matmul(acc, lhsT=w2t[:, f * P:(f + 1) * P],
                                         rhs=g_t[fic],
                                         start=(fic == 0), stop=(fic == FIC - 1))
                fin = sb.tile([P, NQ], F16, tag=f"r{dc}", name=f"fin{dc}")
                nc.vector.scalar_tensor_tensor(
                    fin, acc, ln["bf2"][:, dc:dc + 1], w_t[dc], OP.add, OP.add)
                nc.sync.dma_start(dout_r[:, dc, :], fin)

    nc.compile()
    return nc


def get_nc(debug=False, ec_lim=EC, self_skip=True, emask0=True):
    key = ("nc", debug, ec_lim, self_skip, emask0)
    if key not in _CACHE:
        _CACHE[key] = _build(debug=debug, ec_lim=ec_lim, self_skip=self_skip,
                             emask0=emask0)
    return _CACHE[key]


def _rows(th):
    return np.concatenate(
        [np.arange((2 * j + th) * P, (2 * j + th + 1) * P) for j in range(QC)])


def make_in_maps(dec_inp, enc_out, dec_mask, enc_mask,
                 W_q1, W_kv1, W_o1, g1, b1,
                 W_q2, W_kv2, W_o2, g2, b2,
                 W_ff1, b_ff1, W_ff2, b_ff2, g3, b3,
                 self_skip=True):
    f16 = np.float16
    f32 = np.float32

    def colmajor(v, w):  # [P*w] -> [P, w]
        return np.ascontiguousarray(np.asarray(v, f32).reshape(w, P).T)

    def tile_w(W):
        # [Din, Dout] -> [(Dout/2P)*P, 2*(Din/P)*P]: fc PAIRS interleaved so
        # each partition's DMA line is 4KB contiguous
        di, do = W.shape
        dc, fc = di // P, do // P
        A = (np.asarray(W, f16).reshape(dc, P, fc, P).transpose(2, 1, 0, 3)
             .reshape(fc, P, dc * P))
        return np.ascontiguousarray(
            A.reshape(fc // 2, 2, P, dc * P).transpose(0, 2, 1, 3)
            .reshape(fc // 2 * P, 2 * dc * P))

    def tile_w2(W2):  # [DI, D] -> [DC*2*P, (FIC//2)*P] per (dc, half)
        return np.ascontiguousarray(
            np.asarray(W2, f16).reshape(2, FIC // 2, P, DC, P)
            .transpose(3, 0, 2, 1, 4).reshape(DC * 2 * P, (FIC // 2) * P))

    # LN1/LN3 gain/bias folds (see _build comment): wk1/wv1 rows scaled by
    # g1 (b1 cancels in softmax on the k side; d1 = Wo1^T Wv1^T b1 lands at
    # the Wo1 writer), wff1 rows scaled by g3 with b3 folded into bf1.
    g1f = np.asarray(g1, f32)[:, None]
    g3f = np.asarray(g3, f32)[:, None]
    Wv1 = np.asarray(W_kv1[:, H * DH:], f32)
    d1 = (np.asarray(b1, f32) @ Wv1) @ np.asarray(W_o1, f32)
    bf1f = np.asarray(b_ff1, f32) + np.asarray(b3, f32) @ np.asarray(W_ff1, f32)
    shared = {
        "wq1": tile_w(W_q1),
        "wk1": tile_w(np.asarray(W_kv1[:, :H * DH], f32) * g1f),
        "wv1": np.ascontiguousarray((Wv1 * g1f).astype(f16)),
        "wo1": tile_w(W_o1),
        "wq2": tile_w(W_q2),
        "wk2": tile_w(W_kv2[:, :H * DH]),
        "wv2": np.ascontiguousarray(np.asarray(W_kv2[:, H * DH:], f16)),
        "wo2": tile_w(W_o2),
        "wff1": tile_w(np.asarray(W_ff1, f32) * g3f),
        "wff2": tile_w2(W_ff2),
        "g2": colmajor(g2, DC), "b2": colmajor(b2, DC),
        "bf1": colmajor(bf1f, FIC), "bf2": colmajor(b_ff2, DC),
        "d1": colmajor(d1, DC),
    }
    dec_inp = np.asarray(dec_inp, f32)
    enc_out = np.asarray(enc_out, f32)
    dec_mask = np.asarray(dec_mask)
    enc_mask = np.asarray(enc_mask)
    in_maps = []
    for core in range(8):
        b, th = divmod(core, 2)
        r = _rows(th)
        x_fm = np.ascontiguousarray(dec_inp[:, b, :].T.astype(f16))
        xq_fm = np.ascontiguousarray(dec_inp[r, b, :].T.astype(f16))
        enc_fm = np.ascontiguousarray(enc_out[:, b, :].T.astype(f16))
        if self_skip:
            maskT = np.empty((T, P), f16)
            for kc in range(KC):
                gq = 2 * (kc // 2) + th
                blk = dec_mask[gq * P:(gq + 1) * P, kc * P:(kc + 1) * P, b]
                maskT[kc * P:(kc + 1) * P, :] = np.where(blk.T, f16(0), f16(1))
        else:
            mT = dec_mask[r, :, b].T                  # [T, NQ] bool
            maskT = np.where(mT, f16(0), f16(1))
        emask = np.ascontiguousarray(
            np.where(enc_mask[:, b], -10000.0, 0.0).astype(f32).reshape(EC, P).T)
        in_maps.append(dict(shared, x_fm=x_fm, xq_fm=xq_fm, enc_fm=enc_fm,
                            maskT=maskT, emask=emask))
    return in_maps


def assemble(results):
    out = np.empty((T, B, D), np.float32)
    for core in range(8):
        b, th = divmod(core, 2)
        o = results[core]["out_fm"]
        for j in range(QC):
            g = 2 * j + th
            out[g * P:(g + 1) * P, b, :] = o[:, j * P:(j + 1) * P].T
    return out


def derive_ec_lim(enc_mask):
    """(visible block count, clean) — clean means suffix-padding mask whose
    visible blocks carry zero bias; EC/False (no skip, bias path) otherwise."""
    em = np.asarray(enc_mask)
    nvis = 0
    for b_ in range(em.shape[1]):
        col = em[:, b_]
        first = int(np.argmax(col)) if col.any() else S
        if col[:first].any() or not col[first:].all():
            return EC, False
        nvis = max(nvis, first)
    nb = max(1, min(EC, (nvis + P - 1) // P))
    return nb, nvis % P == 0


def causal_ok(dec_mask):
    dm = np.asarray(dec_mask)
    tri = np.triu(np.ones((T, T), bool), 1)
    return all(np.array_equal(dm[:, :, b_], tri) for b_ in range(dm.shape[2]))


def prepare(inputs):
    self_skip = causal_ok(inputs["dec_mask"])
    ec_lim, clean = derive_ec_lim(inputs["enc_mask"])
    nc = get_nc(ec_lim=ec_lim, self_skip=self_skip, emask0=clean)
    return nc, make_in_maps(**inputs, self_skip=self_skip)


def kernel(**inputs):
    from concourse.bass_utils import run_bass_kernel_spmd

    nc, in_maps = prepare(inputs)
    res = run_bass_kernel_spmd(nc, in_maps, core_ids=list(range(8)))
    return assemble(res.results)



# revision 49
# speedup vs baseline: 1.1154x; 1.1154x over previous
"""Trainium2 Bass kernel for nn_DecoderLayer (self-attn + cross-attn + FFN).

Sharding: 8 cores = 4 batch x 2 interleaved query-block sets (no collectives).
Core (b, th) handles query blocks {2j+th : j=0..3} of batch b, computing
full-sequence K/V locally (feature-major layout, fp16 matmuls, fp32
accumulation; softmax/LN plumbing off the PE).

Structure (v2):
- causal mask applied as a post-exp 0/1 multiply on DVE, trimmed to the one
  diagonal query block per key block (no PE mask matmuls); interleaved query
  blocks balance causal work across the th pair
- attention AV uses V as the stationary operand, producing feature-major
  output directly (no PE transposes); the softmax denominator rides as an
  appended ones-column of V; normalization via DVE reciprocal_approx_fast +
  gpsimd partition_broadcast + DVE multiply
- layernorm stats via DVE tree-adds + gpsimd partition_all_reduce (no PE
  ones-matmuls); rstd via scalar Sqrt + DVE reciprocal_approx_fast
- self-attention emission interleaves cross-attn K/V projection blocks so
  the PE stays busy while the scalar engine grinds exp
- residual stream kept in fp16 (out1/out2), fp16 output DMA
"""

import sys

for _p in ("/opt/trn_rl_repo",):
    if _p not in sys.path:
        sys.path.insert(0, _p)

import numpy as np

import concourse.bass as bass
from concourse import bacc, bass_isa
import concourse.mybir as mybir
import concourse.tile as tile

T, S, B, D, H, DH, DI = 1024, 1024, 4, 1024, 16, 64, 4096
P = 128
NQ = T // 2          # queries per core
DC = D // P          # 8  d chunks
KC = T // P          # 8  key chunks (self)
EC = S // P          # 8  key chunks (cross)
QC = NQ // P         # 4  query chunks
HC = (H * DH) // P   # 8  head-feature chunks
FIC = DI // P        # 32 ffn inner chunks
SCALE = 1.0 / (DH ** 0.5)
EPS = 1e-5

F16 = mybir.dt.float16
F32 = mybir.dt.float32
AF = mybir.ActivationFunctionType
OP = mybir.AluOpType
RED = bass_isa.ReduceOp

_CACHE = {}


def _build(debug=False, ec_lim=EC, self_skip=True, emask0=True):
    nc = bacc.Bacc()

    dx = nc.dram_tensor("x_fm", [D, T], F16, kind="ExternalInput")
    dxq = nc.dram_tensor("xq_fm", [D, NQ], F16, kind="ExternalInput")
    denc = nc.dram_tensor("enc_fm", [D, S], F16, kind="ExternalInput")
    mask_w = P if self_skip else NQ
    dmask = nc.dram_tensor("maskT", [T, mask_w], F16, kind="ExternalInput")
    demask = nc.dram_tensor("emask", [P, EC], F32, kind="ExternalInput")
    # proj_cols weights arrive pre-tiled per output block fc: [n_fc*P, 8*P]
    # (row fc*P+p, col dc*P+q) = W[dc*P+p, fc*P+q]  -> one contiguous 2KB
    # DMA line per partition.  wv1/wv2 stay row-major (proj_v reads rows).
    PAIR = [HC // 2 * P, 2 * DC * P]      # proj_cols pair-tiled weights
    dw = {}
    for nm, shp in [
        ("wq1", PAIR), ("wk1", PAIR), ("wv1", [D, H * DH]),
        ("wo1", PAIR),
        ("wq2", PAIR), ("wk2", PAIR), ("wv2", [D, H * DH]),
        ("wo2", PAIR),
        ("wff1", [FIC // 2 * P, 2 * DC * P]),
        ("wff2", [DC * 2 * P, (FIC // 2) * P]),
    ]:
        dw[nm] = nc.dram_tensor(nm, shp, F16, kind="ExternalInput")
    # g1/b1 and g3/b3 are folded into the adjacent weights host-side:
    # wk1/wv1/wff1 rows are pre-scaled by g, b1's effect on self-attn k
    # cancels in softmax, its effect on v is the fixed vector d1 added at
    # the Wo1 writer, and b3 folds into the gelu bias bf1.
    dln = {}
    for nm, w in [("g2", DC), ("b2", DC), ("bf1", FIC), ("bf2", DC),
                  ("d1", DC)]:
        dln[nm] = nc.dram_tensor(nm, [P, w], F32, kind="ExternalInput")
    dout = nc.dram_tensor("out_fm", [D, NQ], F16, kind="ExternalOutput")

    with tile.TileContext(nc) as tc:
        with tc.tile_pool(name="sb", bufs=1) as sb, \
             tc.tile_pool(name="pp", bufs=2, space="PSUM") as pp:

            # ---------------- constants ----------------
            epsP = sb.tile([P, 1], F32, tag="epsP", name="epsP")
            nc.vector.memset(epsP, EPS)
            ones1 = sb.tile([P, 1], F16, tag="ones1", name="ones1")
            nc.vector.memset(ones1, 1.0)
            onesr = sb.tile([1, P], F16, tag="onesr", name="onesr")
            nc.vector.memset(onesr, 1.0)
            ln = {}
            for nm in dln:
                w = FIC if nm == "bf1" else DC
                ln[nm] = sb.tile([P, w], F32, tag=nm, name=nm)
                nc.sync.dma_start(ln[nm], dln[nm][:, :])
            emask_sb = sb.tile([P, EC], F32, tag="emask", name="emask")
            nc.sync.dma_start(emask_sb, demask[:, :])

            def fam(prefix, n, shape, dtype):
                return [sb.tile(shape, dtype, tag=f"{prefix}{i}", name=f"{prefix}{i}")
                        for i in range(n)]

            def dbg(name, tiles):
                if not debug:
                    return
                w = tiles[0].shape[-1] if len(tiles[0].shape) == 2 else (
                    tiles[0].shape[1] * tiles[0].shape[2])
                dt_ = nc.dram_tensor(name, [len(tiles) * P, w], tiles[0].dtype,
                                     kind="ExternalOutput")
                r = dt_.rearrange("(c p) t -> p c t", p=P)
                for i, t_ in enumerate(tiles):
                    if len(t_.shape) == 3:
                        t_ = t_.rearrange("p a b -> p (a b)")
                    nc.sync.dma_start(r[:, i, :], t_)

            # ---------------- feature-major layernorm ----------------
            def ln_pe(src, blocks, out16, g=None, b=None, name="ln"):
                """LN over the partition (feature) dim; src: DC f16 tiles.

                Stats via PE ones-matmuls (squares on DVE), mean/rstd rows
                broadcast across partitions via K=1 matmuls, short DVE
                apply.  g=None emits (x-m)*rstd only — the gain is folded
                into the consumer's weights host-side.  blocks: list of
                (t0, t1) token windows, each at most 512 wide.
                """
                for j, (jt0, jt1) in enumerate(blocks):
                    js = slice(jt0, jt1)
                    W = jt1 - jt0
                    st = pp.tile([P, 512], F32, tag="avb", name="lns1", bufs=2)
                    st2 = pp.tile([P, 512], F32, tag="avb", name="lns2", bufs=2)
                    for dc in range(DC):
                        sq = sb.tile([P, 512], F16, tag="lnsq", name="lnsq",
                                     bufs=2)
                        nc.vector.tensor_mul(sq[:, :W], src[dc][:, js],
                                             src[dc][:, js])
                        nc.tensor.matmul(st[0:1, :W], lhsT=ones1,
                                         rhs=src[dc][:, js],
                                         start=(dc == 0), stop=(dc == DC - 1))
                        nc.tensor.matmul(st2[0:1, :W], lhsT=ones1,
                                         rhs=sq[:, :W],
                                         start=(dc == 0), stop=(dc == DC - 1))

                    # stat rows as 32-aligned partition-slices of one
                    # scratch tile (slots reused once their reader is done)
                    lr = sb.tile([P, 512], F32, tag="lnr", name="lnr", bufs=2)
                    mm, m2t, var = (lr[0:1, :W], lr[32:33, :W], lr[64:65, :W])
                    nc.vector.tensor_scalar(mm, st[0:1, :W], 1.0 / D, 0.0,
                                            OP.mult, OP.add)
                    nc.vector.tensor_mul(m2t, mm, mm)
                    nc.vector.scalar_tensor_tensor(var, st2[0:1, :W], 1.0 / D,
                                                   m2t, OP.mult, OP.subtract)
                    # rstd = exp(-0.5*ln(var+eps)) — Log/Exp share one ACT
                    # table set (no Sqrt set switch, no DVE reciprocal)
                    lr2 = sb.tile([P, 512], F32, tag="lnr", name="lnr2",
                                  bufs=2)
                    lnv, rstd = lr2[32:33, :W], lr2[0:1, :W]
                    nc.scalar.activation(lnv, var, AF.Ln, bias=epsP[64:65])
                    nc.scalar.activation(rstd, lnv, AF.Exp, scale=-0.5)
                    r16 = sb.tile([1, 512], F16, tag="lnf", name="r16", bufs=2)
                    s16 = sb.tile([1, 512], F16, tag="lnf2", name="s16",
                                  bufs=2)  # -m*rstd (fold) or m (g path)
                    r16, s16 = r16[:, :W], s16[:, :W]
                    nc.vector.tensor_copy(r16, rstd)
                    if g is None:
                        nc.vector.scalar_tensor_tensor(s16, mm, -1.0, rstd,
                                                       OP.mult, OP.mult)
                    else:
                        nc.vector.tensor_copy(s16, mm)
                    bc = pp.tile([P, 1024], F32, tag="sc2", name="lnbc",
                                 bufs=2)
                    nc.tensor.matmul(bc[:, 0:W], lhsT=onesr, rhs=r16,
                                     start=True, stop=True)
                    nc.tensor.matmul(bc[:, 512:512 + W], lhsT=onesr, rhs=s16,
                                     start=True, stop=True)
                    rb = sb.tile([P, 512], F16, tag="lnrb", name="rb", bufs=2)
                    nc.vector.tensor_copy(rb[:, :W], bc[:, 0:W])
                    mb = sb.tile([P, 512], F16, tag="lnmb", name="mb", bufs=2)
                    nc.vector.tensor_copy(mb[:, :W], bc[:, 512:512 + W])
                    rb, mb = rb[:, :W], mb[:, :W]
                    for dc in range(DC):
                        if g is None:
                            t = sb.tile([P, 512], F16, tag="lnt", name="lnt",
                                        bufs=2)
                            nc.vector.tensor_mul(t[:, :W], src[dc][:, js], rb)
                            nc.vector.tensor_add(out16[dc][:, js], t[:, :W],
                                                 mb)
                        else:
                            t = sb.tile([P, 512], F16, tag="lnt", name="lnt",
                                        bufs=2)
                            nc.vector.tensor_sub(t[:, :W], src[dc][:, js], mb)
                            nc.vector.tensor_mul(t[:, :W], t[:, :W], rb)
                            nc.vector.tensor_scalar(
                                out16[dc][:, js], t[:, :W], g[:, dc:dc + 1],
                                b[:, dc:dc + 1], OP.mult, OP.add)

            # ---------------- generic column-block projection ------------
            def proj_cols(wd, rhs, n_fc, writer, rhs_w=NQ, closures=False,
                          jwin=None):
                wr = wd.rearrange("(fp p) x -> fp p x", p=P)
                j0lo, j0hi = jwin if jwin else (0, rhs_w)

                def emit(fp):
                    # one 4KB-per-partition DMA covers output blocks 2fp,2fp+1
                    wt = sb.tile([P, 2 * DC * P], F16, tag="wb", name="wb",
                                 bufs=3)
                    nc.sync.dma_start(wt, wr[fp])
                    for e in range(2):
                        fc = fp * 2 + e
                        for j0 in range(j0lo, j0hi, 512):
                            w_ = min(512, j0hi - j0)
                            js = slice(j0, j0 + w_)
                            acc = pp.tile([P, 512], F32, tag="acc", name="acc",
                                          bufs=2)
                            for dc in range(DC):
                                o = (e * DC + dc) * P
                                nc.tensor.matmul(
                                    acc[:, :w_], lhsT=wt[:, o:o + P],
                                    rhs=rhs[dc][:, js],
                                    start=(dc == 0), stop=(dc == DC - 1))
                            if jwin is None and rhs_w == 512:
                                writer(fc, acc)
                            else:
                                writer(fc, acc[:, :w_], js)

                if closures:
                    return [lambda fp=fp: emit(fp) for fp in range(n_fc // 2)]
                for fp in range(n_fc // 2):
                    emit(fp)

            def proj_v(wd, src, va, ntc=KC, closures=False):
                """token-major V projection (appended ones col), cached W."""
                wr = wd.rearrange("(dc p) f -> p dc f", p=P)
                wvt = []

                def load_w():
                    for dc in range(DC):
                        t = sb.tile([P, H * DH], F16, tag=f"wv{dc}",
                                    name=f"wv{dc}", bufs=1)
                        nc.sync.dma_start(t, wr[:, dc, :])
                        wvt.append(t)

                def emit(tc8):
                    accs = [pp.tile([P, 512], F32, tag="acc", name="acc", bufs=2)
                            for _ in range(2)]
                    for dc in range(DC):
                        for jn in range(2):
                            nc.tensor.matmul(
                                accs[jn],
                                lhsT=src[dc][:, tc8 * P:(tc8 + 1) * P],
                                rhs=wvt[dc][:, jn * 512:(jn + 1) * 512],
                                start=(dc == 0), stop=(dc == DC - 1))
                    for jn in range(2):
                        nc.vector.tensor_copy(
                            va[tc8][:, jn * (H // 2):(jn + 1) * (H // 2), 0:DH],
                            accs[jn].rearrange("p (h d) -> p h d", h=H // 2))
                    nc.gpsimd.memset(va[tc8][:, :, DH:DH + 1], 1.0)

                cl = [load_w] + [(lambda t=t: emit(t)) for t in range(ntc)]
                if closures:
                    return cl
                for c in cl:
                    c()

            # ---------------- attention ----------------
            def attention(qt, kt, va, vec, masked, filler=(), name="sa",
                          qwin=(0, NQ)):
                nkc = KC if masked else ec_lim
                trim = masked and self_skip
                qlo, qhi = qwin
                W = qhi - qlo
                assert not masked or (qlo, qhi) == (0, NQ)

                def q_start(kc):
                    return (kc // 2) * P if trim else 0

                filler = list(filler)
                done = [0]

                def run_filler(i):
                    want = ((i + 1) * len(filler)) // HC
                    while done[0] < want:
                        filler[done[0]]()
                        done[0] += 1

                dbg_av = dbg_den = None
                if debug:
                    dbg_av = nc.dram_tensor(f"dbg_{name}_av", [H * (DH + 1), 512],
                                            F16, kind="ExternalOutput")
                    dbg_den = nc.dram_tensor(f"dbg_{name}_den", [H, 512],
                                             F32, kind="ExternalOutput")

                # narrow (split) windows pack a kc PAIR per head-bank so
                # each bank is filled by two SERIAL same-row-group matmuls
                # (concurrent row-tiled matmuls must not share a psum bank)
                pair = W < 512

                def pcols(hh, q0):
                    # head hh's probs for queries [qlo+q0:qhi) sit at columns
                    # [q0:W) (hh0) / [W:2W-q0) (hh1) — hh1 is shifted left so
                    # the exp span [q0:2W-q0) is contiguous valid
                    return slice(W, 2 * W - q0) if hh else slice(q0, W)

                def prob_ap(pl, kc, hh):
                    if pair:
                        o = hh * 512 + (kc % 2) * W
                        return pl[kc // 2][:, o:o + W]
                    return pl[kc][:, pcols(hh, q_start(kc))]

                def emit_avs(fch, pl):
                    for hh in range(2):
                        h = fch * 2 + hh
                        row = hh * DH
                        av = pp.tile([DH + 1, 512], F32, tag="avb", name="avb",
                                     bufs=2)
                        for kc in range(nkc):
                            q0 = q_start(kc)
                            nc.tensor.matmul(
                                av[:, q0:W], lhsT=va[kc][:, h, :],
                                rhs=prob_ap(pl, kc, hh),
                                start=(kc == 0), stop=(kc == nkc - 1))
                        if debug:
                            cp = sb.tile([DH + 1, 512], F16, tag="dbgav",
                                         name="dbgav", bufs=1)
                            nc.vector.tensor_copy(cp, av)
                            nc.sync.dma_start(
                                dbg_av.rearrange("(h d) q -> h d q", h=H)[h], cp)
                        den_s = sb.tile([1, 512], F32, tag="dens", name="dens",
                                        bufs=2)
                        nc.vector.tensor_copy(den_s[:, :W], av[DH:DH + 1, :W])
                        den = sb.tile([1, 512], F32, tag="den", name="den",
                                      bufs=2)
                        nc.vector.reciprocal_approx_fast(den[:, :W],
                                                         den_s[:, :W])
                        if debug:
                            nc.sync.dma_start(dbg_den[h:h + 1, qlo:qhi],
                                              den[:, :W])
                        denb = sb.tile([DH, 512], F32, tag="denb", name="denb",
                                       bufs=2)
                        nc.gpsimd.partition_broadcast(denb[:, :W], den[:, :W],
                                                      channels=DH)
                        nc.vector.tensor_mul(vec[fch][row:row + DH, qlo:qhi],
                                             av[0:DH, :W], denb[:, :W])

                prev = None
                assert not pair or (not masked and emask0)
                for fch in range(HC):
                    pl = []
                    if pair:
                        for j0 in range(0, nkc, 2):
                            kcs = range(j0, min(j0 + 2, nkc))
                            sp = pp.tile([P, 1024], F32, tag="sc2",
                                         name="sc2", bufs=2)
                            for hh in range(2):
                                row = hh * DH
                                for i, kc in enumerate(kcs):
                                    nc.tensor.matmul(
                                        sp[:, hh * 512 + i * W:
                                           hh * 512 + (i + 1) * W],
                                        lhsT=kt[fch][row:row + DH,
                                                     kc * P:(kc + 1) * P],
                                        rhs=qt[fch][row:row + DH, qlo:qhi],
                                        start=(i == 0),
                                        stop=(i == len(kcs) - 1),
                                        tile_position=(row, 0))
                            pt = sb.tile([P, 1024], F16, tag="p", name="p",
                                         bufs=10)
                            hi = 512 + len(kcs) * W
                            nc.scalar.activation(pt[:, :hi], sp[:, :hi],
                                                 AF.Exp, scale=SCALE)
                            pl.append(pt)
                        if prev is not None:
                            emit_avs(*prev)
                        run_filler(fch)
                        prev = (fch, pl)
                        continue
                    for kc in range(nkc):
                        q0 = q_start(kc)
                        # both heads' scores in one 2-bank PSUM tile so exp
                        # runs as a single wide ACTIVATE (amortizes overhead)
                        sp = pp.tile([P, 1024], F32, tag="sc2", name="sc2",
                                     bufs=2)
                        for hh in range(2):
                            row = hh * DH
                            nc.tensor.matmul(
                                sp[:, pcols(hh, q0)],
                                lhsT=kt[fch][row:row + DH, kc * P:(kc + 1) * P],
                                rhs=qt[fch][row:row + DH, qlo + q0:qhi],
                                start=True, stop=True, tile_position=(row, 0))
                        pt = sb.tile([P, 1024], F16, tag="p", name="p",
                                     bufs=10)
                        if masked:
                            nc.scalar.activation(pt[:, q0:2 * W - q0],
                                                 sp[:, q0:2 * W - q0],
                                                 AF.Exp, scale=SCALE)
                            if trim:
                                for hh in range(2):
                                    o = hh * 512 + (q0 if hh == 0 else 0)
                                    nc.vector.tensor_mul(
                                        pt[:, o:o + P], pt[:, o:o + P],
                                        mask_sb[kc])
                            else:
                                for hh in range(2):
                                    o = hh * 512
                                    nc.vector.tensor_mul(
                                        pt[:, o:o + 512], pt[:, o:o + 512],
                                        mask_sb[kc])
                        elif emask0:
                            nc.scalar.activation(pt[:, :2 * W], sp[:, :2 * W],
                                                 AF.Exp, scale=SCALE)
                        else:
                            for hh in range(2):
                                o = hh * W
                                nc.scalar.activation(
                                    pt[:, o:o + W], sp[:, o:o + W], AF.Exp,
                                    bias=emask_sb[:, kc:kc + 1], scale=SCALE)
                        pl.append(pt)
                    if prev is not None:
                        emit_avs(*prev)
                    run_filler(fch)
                    prev = (fch, pl)
                emit_avs(*prev)

            # ================ phase A: load x, q1, LN1 ================
            xq_t = fam("q", DC, [P, NQ], F16)        # xq (q-proj rhs + residual)
            dxq_r = dxq.rearrange("(dc p) t -> p dc t", p=P)
            for dc in range(DC):
                nc.sync.dma_start(xq_t[dc], dxq_r[:, dc, :])

            t_t = fam("t", HC, [P, NQ], F16)         # q1, later h2_h
            # q1 only needs xq — emit before LN1 so PE is busy during LN1;
            # x is DMAed after q1's weights so q1 starts ASAP
            proj_cols(dw["wq1"], xq_t, HC,
                      lambda fc, acc: nc.vector.tensor_copy(t_t[fc], acc))

            e_t = fam("e", DC, [P, T], F16)          # x, then c, later enc
            dx_r = dx.rearrange("(dc p) t -> p dc t", p=P)
            for dc in range(DC):
                nc.sync.dma_start(e_t[dc], dx_r[:, dc, :])

            mask_sb = []
            dmask_r = dmask.rearrange("(kc p) q -> p kc q", p=P)
            for kc in range(KC):
                mt = sb.tile([P, mask_w], F16, tag=f"m{kc}", name=f"m{kc}")
                nc.sync.dma_start(mt, dmask_r[:, kc, :])
                mask_sb.append(mt)

            ln_pe(e_t, [(0, 512), (512, 1024)], out16=e_t, name='ln1')   # c' = (x-m)*rstd in e_t (g1 folded)
            dbg("dbg_c", e_t)

            # ================ phase B: self-attn K/V ================
            k_t = fam("k", HC, [P, T], F16)          # k1, later k2
            va_t = fam("va", KC, [P, H, DH + 1], F16)
            proj_cols(dw["wk1"], e_t, HC,
                      lambda fc, acc, js: nc.vector.tensor_copy(k_t[fc][:, js], acc),
                      rhs_w=T)
            proj_v(dw["wv1"], e_t, va_t)
            dbg("dbg_q1", t_t)
            dbg("dbg_k1", k_t)
            dbg("dbg_va", va_t)

            # enc + cross K/V are independent of self-attn; their projection
            # blocks run as PE filler between self-attn head groups.
            e2_t = fam("e", DC, [P, S], F16)
            denc_r = denc.rearrange("(dc p) t -> p dc t", p=P)
            for dc in range(DC):
                nc.sync.dma_start(e2_t[dc], denc_r[:, dc, :])
            k2_t = fam("k", HC, [P, S], F16)
            va2_t = fam("va", EC, [P, H, DH + 1], F16)
            k2_cl = proj_cols(
                dw["wk2"], e2_t, HC,
                lambda fc, acc, js: nc.vector.tensor_copy(k2_t[fc][:, js], acc),
                rhs_w=ec_lim * P, closures=True)
            v2_cl = proj_v(dw["wv2"], e2_t, va2_t, ntc=ec_lim, closures=True)
            # K2 (+ the V2 weight DMA) fills self-attention; the V2 emits run
            # later, under LN2/q2 where the PE would otherwise idle
            filler = [v2_cl[0]] + k2_cl

            # ================ phase C: self-attention ================
            vec_t = fam("s", HC, [P, NQ], F16)       # vec1, later vec2, h3
            attention(t_t, k_t, va_t, vec_t, masked=True, filler=filler, name="sa")
            dbg("dbg_vec", vec_t)

            # ================ phase D: Wo1 + residual, LN2 ================
            r_t = fam("r", DC, [P, NQ], F16)         # out1 (f16 residual src)
            proj_cols(dw["wo1"], vec_t, DC,
                      lambda fc, acc: nc.vector.scalar_tensor_tensor(
                          r_t[fc], acc, ln["d1"][:, fc:fc + 1], xq_t[fc],
                          OP.add, OP.add))
            for c in v2_cl[1:]:
                c()
            dbg("dbg_out1", r_t)

            h2h_t = fam("t", HC, [P, NQ], F16)       # reuse t family
            ln_pe(r_t, [(0, 512)], out16=h2h_t, g=ln["g2"], b=ln["b2"],
                  name='ln2')
            dbg("dbg_h2", h2h_t)

            # ====== phase E/F: cross-attention || LN3+FFN1 (query halves),
            # ====== then full-width FFN2
            q2_t = fam("q", HC, [P, NQ], F16)        # reuse q family
            proj_cols(dw["wq2"], h2h_t, HC,
                      lambda fc, acc: nc.vector.tensor_copy(q2_t[fc], acc))

            vec2_t = fam("s", HC, [P, NQ], F16)
            w_t = fam("w", DC, [P, NQ], F16)         # out2
            h3_t = fam("r", DC, [P, NQ], F16)        # reuse out1's slots
            g_t = fam("gg", FIC, [P, NQ], F16)       # (dead after LN2)
            # NOTE: a (0,256)/(256,512) query split overlapping CA with the
            # FFN measured SLOWER (ACT table-set thrash between interleaved
            # exp and gelu calls: 43 table loads, +55us) — keep full width.
            halves = ((0, NQ),)
            for t0, t1 in halves:
                attention(q2_t, k2_t, va2_t, vec2_t, masked=False,
                          name=f"ca{t0}", qwin=(t0, t1))
                proj_cols(dw["wo2"], vec2_t, DC,
                          lambda fc, acc, js: nc.vector.tensor_add(
                              w_t[fc][:, js], acc, h2h_t[fc][:, js]),
                          jwin=(t0, t1))
                ln_pe(w_t, [(t0, t1)], out16=h3_t, name=f'ln3_{t0}')
                proj_cols(dw["wff1"], h3_t, FIC,
                          lambda fc, acc, js: nc.scalar.activation(
                              g_t[fc][:, js], acc, AF.Gelu,
                              bias=ln["bf1"][:, fc:fc + 1], scale=1.0),
                          jwin=(t0, t1))
            dbg("dbg_vec2", vec2_t)
            dbg("dbg_out2", w_t)

            dout_r = dout.rearrange("(dc p) q -> p dc q", p=P)
            w2r = dw["wff2"].rearrange("(dc half p) x -> dc half p x",
                                       half=2, p=P)
            for dc in range(DC):
                acc = pp.tile([P, 512], F32, tag="acc", name="acc", bufs=2)
                for half in range(2):
                    w2t = sb.tile([P, (FIC // 2) * P], F16, tag="wf2",
                                  name="wf2", bufs=2)
                    nc.sync.dma_start(w2t, w2r[dc, half])
                    for f in range(FIC // 2):
                        fic = half * (FIC // 2) + f
                        nc.tensor.matmul(acc, lhsT=w2t[:, f * P:(f + 1) * P],
                                         rhs=g_t[fic],
                                         start=(fic == 0), stop=(fic == FIC - 1))
                fin = sb.tile([P, NQ], F16, tag=f"r{dc}", name=f"fin{dc}")
                nc.vector.scalar_tensor_tensor(
                    fin, acc, ln["bf2"][:, dc:dc + 1], w_t[dc], OP.add, OP.add)
                nc.sync.dma_start(dout_r[:, dc, :], fin)

    nc.compile()
    return nc


def get_nc(debug=False, ec_lim=EC, self_skip=True, emask0=True):
    key = ("nc", debug, ec_lim, self_skip, emask0)
    if key not in _CACHE:
        _CACHE[key] = _build(debug=debug, ec_lim=ec_lim, self_skip=self_skip,
                             emask0=emask0)
    return _CACHE[key]


def _rows(th):
    return np.concatenate(
        [np.arange((2 * j + th) * P, (2 * j + th + 1) * P) for j in range(QC)])


def make_in_maps(dec_inp, enc_out, dec_mask, enc_mask,
                 W_q1, W_kv1, W_o1, g1, b1,
                 W_q2, W_kv2, W_o2, g2, b2,
                 W_ff1, b_ff1, W_ff2, b_ff2, g3, b3,
                 self_skip=True):
    f16 = np.float16
    f32 = np.float32

    def colmajor(v, w):  # [P*w] -> [P, w]
        return np.ascontiguousarray(np.asarray(v, f32).reshape(w, P).T)

    def tile_w(W):
        # [Din, Dout] -> [(Dout/2P)*P, 2*(Din/P)*P]: fc PAIRS interleaved so
        # each partition's DMA line is 4KB contiguous
        di, do = W.shape
        dc, fc = di // P, do // P
        A = (np.asarray(W, f16).reshape(dc, P, fc, P).transpose(2, 1, 0, 3)
             .reshape(fc, P, dc * P))
        return np.ascontiguousarray(
            A.reshape(fc // 2, 2, P, dc * P).transpose(0, 2, 1, 3)
            .reshape(fc // 2 * P, 2 * dc * P))

    def tile_w2(W2):  # [DI, D] -> [DC*2*P, (FIC//2)*P] per (dc, half)
        return np.ascontiguousarray(
            np.asarray(W2, f16).reshape(2, FIC // 2, P, DC, P)
            .transpose(3, 0, 2, 1, 4).reshape(DC * 2 * P, (FIC // 2) * P))

    # LN1/LN3 gain/bias folds (see _build comment): wk1/wv1 rows scaled by
    # g1 (b1 cancels in softmax on the k side; d1 = Wo1^T Wv1^T b1 lands at
    # the Wo1 writer), wff1 rows scaled by g3 with b3 folded into bf1.
    g1f = np.asarray(g1, f32)[:, None]
    g3f = np.asarray(g3, f32)[:, None]
    Wv1 = np.asarray(W_kv1[:, H * DH:], f32)
    d1 = (np.asarray(b1, f32) @ Wv1) @ np.asarray(W_o1, f32)
    bf1f = np.asarray(b_ff1, f32) + np.asarray(b3, f32) @ np.asarray(W_ff1, f32)
    shared = {
        "wq1": tile_w(W_q1),
        "wk1": tile_w(np.asarray(W_kv1[:, :H * DH], f32) * g1f),
        "wv1": np.ascontiguousarray((Wv1 * g1f).astype(f16)),
        "wo1": tile_w(W_o1),
        "wq2": tile_w(W_q2),
        "wk2": tile_w(W_kv2[:, :H * DH]),
        "wv2": np.ascontiguousarray(np.asarray(W_kv2[:, H * DH:], f16)),
        "wo2": tile_w(W_o2),
        "wff1": tile_w(np.asarray(W_ff1, f32) * g3f),
        "wff2": tile_w2(W_ff2),
        "g2": colmajor(g2, DC), "b2": colmajor(b2, DC),
        "bf1": colmajor(bf1f, FIC), "bf2": colmajor(b_ff2, DC),
        "d1": colmajor(d1, DC),
    }
    dec_inp = np.asarray(dec_inp, f32)
    enc_out = np.asarray(enc_out, f32)
    dec_mask = np.asarray(dec_mask)
    enc_mask = np.asarray(enc_mask)
    in_maps = []
    for core in range(8):
        b, th = divmod(core, 2)
        r = _rows(th)
        x_fm = np.ascontiguousarray(dec_inp[:, b, :].T.astype(f16))
        xq_fm = np.ascontiguousarray(dec_inp[r, b, :].T.astype(f16))
        enc_fm = np.ascontiguousarray(enc_out[:, b, :].T.astype(f16))
        if self_skip:
            maskT = np.empty((T, P), f16)
            for kc in range(KC):
                gq = 2 * (kc // 2) + th
                blk = dec_mask[gq * P:(gq + 1) * P, kc * P:(kc + 1) * P, b]
                maskT[kc * P:(kc + 1) * P, :] = np.where(blk.T, f16(0), f16(1))
        else:
            mT = dec_mask[r, :, b].T                  # [T, NQ] bool
            maskT = np.where(mT, f16(0), f16(1))
        emask = np.ascontiguousarray(
            np.where(enc_mask[:, b], -10000.0, 0.0).astype(f32).reshape(EC, P).T)
        in_maps.append(dict(shared, x_fm=x_fm, xq_fm=xq_fm, enc_fm=enc_fm,
                            maskT=maskT, emask=emask))
    return in_maps


def assemble(results):
    out = np.empty((T, B, D), np.float32)
    for core in range(8):
        b, th = divmod(core, 2)
        o = results[core]["out_fm"]
        for j in range(QC):
            g = 2 * j + th
            out[g * P:(g + 1) * P, b, :] = o[:, j * P:(j + 1) * P].T
    return out


def derive_ec_lim(enc_mask):
    """(visible block count, clean) — clean means suffix-padding mask whose
    visible blocks carry zero bias; EC/False (no skip, bias path) otherwise."""
    em = np.asarray(enc_mask)
    nvis = 0
    for b_ in range(em.shape[1]):
        col = em[:, b_]
        first = int(np.argmax(col)) if col.any() else S
        if col[:first].any() or not col[first:].all():
            return EC, False
        nvis = max(nvis, first)
    nb = max(1, min(EC, (nvis + P - 1) // P))
    return nb, nvis % P == 0


def causal_ok(dec_mask):
    dm = np.asarray(dec_mask)
    tri = np.triu(np.ones((T, T), bool), 1)
    return all(np.array_equal(dm[:, :, b_], tri) for b_ in range(dm.shape[2]))


def prepare(inputs):
    self_skip = causal_ok(inputs["dec_mask"])
    ec_lim, clean = derive_ec_lim(inputs["enc_mask"])
    nc = get_nc(ec_lim=ec_lim, self_skip=self_skip, emask0=clean)
    return nc, make_in_maps(**inputs, self_skip=self_skip)


def kernel(**inputs):
    from concourse.bass_utils import run_bass_kernel_spmd

    nc, in_maps = prepare(inputs)
    res = run_bass_kernel_spmd(nc, in_maps, core_ids=list(range(8)))
    return assemble(res.results)

